# revision 39
# baseline (speedup 1.0000x reference)
"""Trainium2 Bass kernel for causal multi-head attention (b=2, n=2048, d=1024, h=16).

Sharding: 8 cores; core c handles batch (c // 4) and the 4 heads
[4*(c%4), 4*(c%4)+4).  Each core computes its heads' attention plus its
partial output projection y_part = O_heads @ Wo[:, cols].T ; the host sums
the 4 per-batch partials and adds bo + bv @ Wo.T (the V-bias contribution
commutes with the softmax average, so it never goes on device).

All matmuls run in bf16 (measured: bf16@512 issues at ~259 ns vs f32r's
~285-500 ns, and narrow f32r matmuls fall off a cliff).  fp32 PSUM
accumulation keeps the error at ~3.5e-3 (CPU-simulated) vs the 2e-2 gate.

On-device pipeline per core, interleaved per 512-token block:
  xT (d-major, bf16) -> QT,KT [hd, n] and V [n, hd] projections
  ST tile [k,q] = KT-chunk.T x QTz_h      (bf16, K=128: both heads' KT rows
                                           with the other head's QT rows
                                           zeroed; 1/8 scale folded into Wq)
  PT = exp(ST) in bf16 (no max subtraction; fp32-exp safe)
  causal masking by multiplying diagonal-band tiles with a bf16 0/1 mask
  OT [h%2 ? 64:128 rows, q] += V_aug-chunk.T x PT ; V_aug carries a ones
      column so the softmax denominator l lands at partition 64 (even
      heads) / partition 0 (odd heads), next to OT's 64 rows.
  rl = 1/l (one-row DVE reciprocal), broadcast across partitions with a
      single SBUF->SBUF DMA (no DRAM bounce)
  OTall[(h%2)*64:+64, h//2, q] = OT * rl   (pair-packed, bf16)
  y[tok, :] += OTall-pair-chunk.T x WoP    (bf16, K=128 dense: head pairs
      stacked, so no zero-row padding in the output projection)
"""

import numpy as np

import concourse.bass as bass
import concourse.mybir as mybir
import concourse.tile as tile
from concourse import bacc
from concourse.bass_utils import run_bass_kernel_spmd

D = 1024          # d_model
N = 2048          # sequence length
B = 2             # batch
H_TOT = 16        # total heads
HD = 64           # head dim
HPC = 4           # heads per core
NCORES = 8
SCALE = HD ** -0.5

F32 = mybir.dt.float32
BF16 = mybir.dt.bfloat16

QTILE = 512       # q-tile width (free dim of score matmuls)
KCH = 128         # k-chunk (partition dim of score tiles)
NQT = N // QTILE  # 4
NKC = N // KCH    # 16
DCH = D // 128    # 8 d_model chunks
NJUNK = 18        # PE warm-up matmuls
DEBUG_DUMP = False  # add debug DRAM outputs (OTall, l, 1/l)


def build_kernel():
    nc = bacc.Bacc("TRN2", target_bir_lowering=False, debug=False,
                   num_devices=NCORES)

    # inputs pre-tiled on host into partition-major layouts so each load is
    # one DMA with fat (multi-KB) per-partition contiguous descriptors
    xb0 = nc.dram_tensor("xb0", [128, DCH * QTILE], BF16,
                         kind="ExternalInput").ap()
    xrest = nc.dram_tensor("xrest", [128, DCH * 3 * QTILE], BF16,
                           kind="ExternalInput").ap()
    wqk = nc.dram_tensor("wqk", [128, 2 * DCH * 256], BF16,
                         kind="ExternalInput").ap()
    wvo = nc.dram_tensor("wvo", [128, DCH * 256 + 2 * D], BF16,
                         kind="ExternalInput").ap()
    bqz = nc.dram_tensor("bqz", [128, HPC], F32, kind="ExternalInput").ap()
    sclz = nc.dram_tensor("sclz", [128, HPC], F32, kind="ExternalInput").ap()
    bkd = nc.dram_tensor("bk", [HPC * HD], F32, kind="ExternalInput").ap()
    maskd = nc.dram_tensor("mask", [128, 896], BF16, kind="ExternalInput").ap()
    y = nc.dram_tensor("y", [N, D], F32, kind="ExternalOutput").ap()
    if DEBUG_DUMP:
        dbg_ot = nc.dram_tensor("dbg_ot", [128, 2 * N], BF16,
                                kind="ExternalOutput").ap()
        dbg_l = nc.dram_tensor("dbg_l", [16, 512], F32,
                               kind="ExternalOutput").ap()
        dbg_rl = nc.dram_tensor("dbg_rl", [16, 512], F32,
                                kind="ExternalOutput").ap()
        dbg_xk = nc.dram_tensor("dbg_xk", [128, 8 * 512], BF16,
                                kind="ExternalOutput").ap()
        dbg_qt = nc.dram_tensor("dbg_qt", [128, 2 * 512], BF16,
                                kind="ExternalOutput").ap()
        dbg_kt = nc.dram_tensor("dbg_kt", [128, 4 * 512], BF16,
                                kind="ExternalOutput").ap()

    Exp = mybir.ActivationFunctionType.Exp
    Identity = mybir.ActivationFunctionType.Identity

    with tile.TileContext(nc) as tc:
        from contextlib import ExitStack
        with ExitStack() as ctx:
            singles = ctx.enter_context(tc.tile_pool(name="singles", bufs=1))
            pt_pool = ctx.enter_context(tc.tile_pool(name="pt", bufs=4))
            r_pool = ctx.enter_context(tc.tile_pool(name="rp", bufs=2))
            yout = ctx.enter_context(tc.tile_pool(name="yout", bufs=3))
            dram = ctx.enter_context(
                tc.tile_pool(name="dram", bufs=2, space="DRAM"))
            ps_mm = ctx.enter_context(
                tc.tile_pool(name="psmm", bufs=2, space="PSUM"))
            ps_st = ctx.enter_context(
                tc.tile_pool(name="psst", bufs=2, space="PSUM"))
            ps_ot = ctx.enter_context(
                tc.tile_pool(name="psot", bufs=2, space="PSUM"))

            # --- resident inputs (DMA order = priority: first MMs need
            # wq/wk + x block 0) -------------------------------------------
            wqk_sb = singles.tile([128, 2, DCH, 256], BF16)
            nc.sync.dma_start(
                wqk_sb[:], wqk.rearrange("p (t o m) -> p t o m", t=2, o=DCH))
            wq_sb = wqk_sb[:, 0]
            wk_sb = wqk_sb[:, 1]
            bqz_sb = singles.tile([128, HPC], F32)
            nc.sync.dma_start(bqz_sb[:], bqz)
            sclz_sb = singles.tile([128, HPC], F32)
            nc.sync.dma_start(sclz_sb[:], sclz)
            bk_sb = singles.tile([128, 2], F32)
            nc.sync.dma_start(bk_sb[:], bkd.rearrange("(o p) -> p o", p=128))

            # x lives in one [128, k-chunk, token] tile; block 0 lands first
            # so compute can start early.
            XKA = singles.tile([128, DCH, N], BF16, name="xka")
            xk = [[XKA[:, k, b * QTILE:(b + 1) * QTILE] for b in range(NQT)]
                  for k in range(DCH)]
            nc.sync.dma_start(
                XKA[:, :, 0:QTILE],
                xb0.rearrange("p (o m) -> p o m", o=DCH))

            wvo_sb = singles.tile([128, DCH * 256 + 2 * D], BF16)
            nc.sync.dma_start(wvo_sb[:], wvo)
            wv_sb = wvo_sb[:, :DCH * 256].rearrange("p (o m) -> p o m", o=DCH)
            wop_sb = wvo_sb[:, DCH * 256:].rearrange("p (t m) -> p t m", t=2)

            nc.sync.dma_start(
                XKA[:, :, QTILE:],
                xrest.rearrange("p (o m) -> p o m", o=DCH))
            mask_sb = singles.tile([128, 896], BF16)
            nc.sync.dma_start(mask_sb[:], maskd)

            # PE warm-up: the first ~8us are DMA-bound with the PE idle,
            # which leaves the PE clock throttled to 1.2 GHz when real work
            # starts.  Issue dependency-free junk matmuls so the activity
            # monitor unthrottles before the first projection matmul.
            junk = singles.tile([128, 512], BF16)
            nc.vector.memset(junk[:], 0.0)
            ones1 = singles.tile([1, 128], BF16, name="ones1")
            nc.vector.memset(ones1[:], 1.0)
            for i in range(NJUNK):
                wps = ps_ot.tile([128, 512], F32, tag="ot", name="wps")
                nc.tensor.matmul(wps[:], lhsT=junk[:, :128], rhs=junk[:],
                                 start=True, stop=True)

            # V_aug layout per head (128 cols each inside V_sb):
            #   even h: cols 0-63 = V, col 64 = ones  -> l at partition 64
            #   odd h:  col 0 = ones, cols 64-127 = V -> l at partition 0
            # (cols 65-127 even / 1-63 odd zeroed so stray PSUM rows stay 0)
            QTz = [[singles.tile([128, QTILE], BF16, name=f"qtz{h}_{i}")
                    for i in range(NQT)] for h in range(HPC)]
            KT_sb = [singles.tile([128, 2, QTILE], BF16, name=f"kt{i}")
                     for i in range(NQT)]
            V_sb = [singles.tile([128, 4, HPC * 128], BF16, name=f"v{i}")
                    for i in range(NQT)]
            OTall = singles.tile([128, 2, N], BF16, name="otall")
            for blk in range(NQT):
                nc.gpsimd.memset(V_sb[blk][:], 0.0)
                for h in range(HPC):
                    oc = h * 128 + (64 if h % 2 == 0 else 0)
                    nc.gpsimd.memset(V_sb[blk][:, :, oc:oc + 1], 1.0)

            def project(blk):
                for m in range(2):
                    ps = ps_mm.tile([128, 512], F32, tag="mm")
                    for k in range(DCH):
                        nc.tensor.matmul(
                            ps[:],
                            lhsT=wq_sb[:, k, m * 128:(m + 1) * 128],
                            rhs=xk[k][blk][:],
                            start=(k == 0), stop=(k == DCH - 1))
                    for hh in range(2):
                        h = 2 * m + hh
                        # per-partition scale zeroes the other head's rows
                        nc.scalar.activation(
                            QTz[h][blk][:], ps[:], Identity,
                            bias=bqz_sb[:, h:h + 1],
                            scale=sclz_sb[:, h:h + 1])
                for m in range(2):
                    ps = ps_mm.tile([128, 512], F32, tag="mm")
                    for k in range(DCH):
                        nc.tensor.matmul(
                            ps[:],
                            lhsT=wk_sb[:, k, m * 128:(m + 1) * 128],
                            rhs=xk[k][blk][:],
                            start=(k == 0), stop=(k == DCH - 1))
                    nc.scalar.activation(
                        KT_sb[blk][:, m, :], ps[:], Identity,
                        bias=bk_sb[:, m:m + 1], scale=1.0)
                for tt in range(4):
                    ps = ps_mm.tile([128, 512], F32, tag="mm")
                    for k in range(DCH):
                        nc.tensor.matmul(
                            ps[:, :HPC * HD],
                            lhsT=xk[k][blk][:, tt * 128:(tt + 1) * 128],
                            rhs=wv_sb[:, k, :],
                            start=(k == 0), stop=(k == DCH - 1))
                    for h in range(HPC):
                        vc = h * 128 + (0 if h % 2 == 0 else 64)
                        # split casts across ACT and DVE so neither queue
                        # rate-limits the projection PSUM rotation
                        if (tt + h) % 2 == 0:
                            nc.scalar.activation(
                                V_sb[blk][:, tt, vc:vc + HD],
                                ps[:, h * HD:(h + 1) * HD], Identity)
                        else:
                            nc.vector.tensor_copy(
                                V_sb[blk][:, tt, vc:vc + HD],
                                ps[:, h * HD:(h + 1) * HD])

            def attend(qi, carry):
                # carry: pending normalize-finish thunks (recip+mul) whose
                # broadcast DMAs are still in flight.  Each is flushed two
                # heads later so the DVE never stalls on the DMA bounce.
                q0 = qi * QTILE
                for h in range(HPC):
                    mi = h // 2
                    po = (h % 2) * 64          # OT partition offset
                    vbase = h * 128
                    nrows = 65 if h % 2 == 0 else 128
                    lrow = 64 if h % 2 == 0 else 0
                    nprs = 2 * (qi + 1)        # pairs of 128-k-chunks
                    pso = ps_ot.tile([128, 512], F32, tag="ot", name="pso")

                    pair_ps = [None] * nprs
                    pair_pt = [None] * nprs

                    def consume(pi):
                        # last pair of each q-tile: only columns q >= r are
                        # unmasked (r = 256, 384); compute just those.
                        shrunk = (pi == nprs - 1)
                        pss, pt = pair_ps[pi], pair_pt[pi]
                        if shrunk:
                            for j in range(2):
                                r = (2 * pi + j) * KCH - q0
                                nc.scalar.activation(
                                    pt[:, j, r:], pss[:, j, r:], Exp)
                        else:
                            nc.scalar.activation(pt[:], pss[:], Exp)
                        for j in range(2):
                            ki = 2 * pi + j
                            r = ki * KCH - q0
                            if r >= 0:
                                s = max(r, 0)
                                nc.vector.tensor_mul(
                                    pt[:, j, s:], pt[:, j, s:],
                                    mask_sb[:, 384 - r + s:384 - r + 512])
                        for j in range(2):
                            ki = 2 * pi + j
                            s = max(ki * KCH - q0, 0)
                            nc.tensor.matmul(
                                pso[:nrows, s:],
                                lhsT=V_sb[ki // 4][:, ki % 4,
                                                   vbase:vbase + nrows],
                                rhs=pt[:, j, s:],
                                start=(ki == 0), stop=(ki == 4 * (qi + 1) - 1))

                    for pi in range(nprs):
                        pss = ps_st.tile([128, 2, 512], F32, tag="st",
                                         name="pss")
                        pair_ps[pi] = pss
                        pair_pt[pi] = pt_pool.tile([128, 2, 512], BF16,
                                                   tag="pt", name="pt")
                        for j in range(2):
                            ki = 2 * pi + j
                            s = (ki * KCH - q0) if pi == nprs - 1 else 0
                            nc.tensor.matmul(
                                pss[:, j, s:],
                                lhsT=KT_sb[ki // 4][:, mi,
                                                    (ki % 4) * 128:
                                                    (ki % 4) * 128 + 128],
                                rhs=QTz[h][qi][:, s:],
                                start=True, stop=True)
                        if pi > 0:
                            consume(pi - 1)
                    consume(nprs - 1)

                    # normalize, pipelined: drain now (stage OT+l off PSUM,
                    # 1/l bounce through DRAM — SBUF DMA sources reject zero
                    # partition stride), finish (recip+mul) two heads later
                    # so the DVE queue never waits on the bounce round trip.
                    # reciprocal ops must START at partition 0 (offset-64
                    # reciprocal returns garbage on HW), so the recip-vs-
                    # broadcast order flips with head parity.
                    if len(carry) == 2:
                        carry.pop(0)()
                    otu = r_pool.tile([128, 512], F32, tag="otu", name="otu")
                    nc.vector.tensor_copy(otu[:nrows, :], pso[:nrows, :])
                    sc = dram.tile([1, 512], F32, tag="sc", name="sc")
                    rb = r_pool.tile([128, 512], F32, tag="rb", name="rb")
                    if h % 2 == 0:
                        # l at partition 64: broadcast raw l, then recip the
                        # 64 partitions we need (offset 0).
                        nc.sync.dma_start(sc[:], otu[lrow:lrow + 1, :])
                        row = sc[0, :]
                        bcast = bass.AP(tensor=row.tensor, offset=row.offset,
                                        ap=[[0, HD]] + list(row.ap))
                        nc.sync.dma_start(rb[:HD, :], bcast)

                        def finish(otu=otu, rb=rb, po=po, mi=mi):
                            nc.vector.reciprocal_approx_fast(
                                out=rb[:HD, :], in_=rb[:HD, :])
                            nc.vector.tensor_mul(
                                OTall[po:po + HD, mi, q0:q0 + QTILE],
                                otu[po:po + HD, :], rb[po:po + HD, :])
                    elif qi == NQT - 1 and h == HPC - 1:
                        # very last head: the DMA bounce's ~5us latency would
                        # sit on the critical path before the final out-proj.
                        # Broadcast 1/l with a K=1 ones-matmul on the PE and
                        # drain it through ACT instead (~1.5us).
                        rl = r_pool.tile([128, 512], F32, tag="rl", name="rl")
                        nc.vector.reciprocal_approx_fast(
                            out=rl[0:1, :], in_=otu[0:1, :])
                        rlb = r_pool.tile([128, 512], BF16, tag="rlb",
                                          name="rlb")
                        nc.vector.tensor_copy(rlb[0:1, :], rl[0:1, :])
                        rb_ps = ps_mm.tile([128, 512], F32, tag="mm",
                                           name="rbps")
                        nc.tensor.matmul(rb_ps[:], lhsT=ones1[0:1, :],
                                         rhs=rlb[0:1, :], start=True,
                                         stop=True)
                        nc.scalar.activation(rb[:], rb_ps[:], Identity)

                        def finish(otu=otu, rb=rb, po=po, mi=mi):
                            nc.vector.tensor_mul(
                                OTall[po:po + HD, mi, q0:q0 + QTILE],
                                otu[po:po + HD, :], rb[po:po + HD, :])
                    else:
                        # l at partition 0: recip the single row first, then
                        # broadcast 1/l.
                        rl = r_pool.tile([128, 512], F32, tag="rl", name="rl")
                        nc.vector.reciprocal_approx_fast(
                            out=rl[0:1, :], in_=otu[0:1, :])
                        nc.sync.dma_start(sc[:], rl[0:1, :])
                        row = sc[0, :]
                        bcast = bass.AP(tensor=row.tensor, offset=row.offset,
                                        ap=[[0, HD]] + list(row.ap))
                        nc.sync.dma_start(rb[HD:, :], bcast)

                        def finish(otu=otu, rb=rb, po=po, mi=mi):
                            nc.vector.tensor_mul(
                                OTall[po:po + HD, mi, q0:q0 + QTILE],
                                otu[po:po + HD, :], rb[po:po + HD, :])
                    carry.append(finish)
                    if DEBUG_DUMP:
                        di = h * 4 + qi
                        nc.sync.dma_start(dbg_l[di:di + 1, :],
                                          otu[lrow:lrow + 1, :])
                        nc.sync.dma_start(dbg_rl[di:di + 1, :],
                                          rb[po:po + 1, :])

            def outproj(qi):
                # output projection for this q-block's 4 token chunks
                q0 = qi * QTILE
                for tt in range(4):
                    t0 = q0 + tt * 128
                    for half in range(2):
                        # outproj runs between project(qi+1) and attend(qi+1)
                        # when the score-psum pool is idle — borrow it so the
                        # rotation never waits on projection PSUM drains.
                        ps = ps_st.tile([128, 512], F32, tag="st",
                                        name="psy")
                        for m in range(2):
                            nc.tensor.matmul(
                                ps[:],
                                lhsT=OTall[:, m, t0:t0 + 128],
                                rhs=wop_sb[:, m, half * 512:half * 512 + 512],
                                start=(m == 0), stop=(m == 1))
                        yt = yout.tile([128, 512], F32, tag="y", name="yt")
                        # split drain copies across DVE and ACT so neither
                        # engine rate-limits the out-proj PSUM rotation
                        if half == 0:
                            nc.vector.tensor_copy(yt[:], ps[:])
                        else:
                            nc.scalar.activation(yt[:], ps[:], Identity)
                        nc.sync.dma_start(
                            y[t0:t0 + 128, half * 512:half * 512 + 512],
                            yt[:])

            # software pipeline: emit outproj(qi) after project(qi+1) so the
            # PE covers the last head's normalize latency (DMA bounce) with
            # the next block's projection matmuls instead of stalling.
            pend = []
            for blk in range(NQT):
                project(blk)
                if blk > 0:
                    for f in pend:
                        f()
                    pend = []
                    outproj(blk - 1)
                attend(blk, pend)
            for f in pend:
                f()
            outproj(NQT - 1)
            if DEBUG_DUMP:
                nc.sync.dma_start(
                    dbg_ot.rearrange("p (t m) -> p t m", t=2), OTall[:])
                dxk = dbg_xk.rearrange("p (i m) -> p i m", i=8)
                for i, (k, b) in enumerate(
                        [(0, 2), (0, 3), (3, 2), (3, 3),
                         (5, 2), (5, 3), (7, 2), (7, 3)]):
                    nc.sync.dma_start(dxk[:, i, :], xk[k][b][:])
                dqt = dbg_qt.rearrange("p (i m) -> p i m", i=2)
                nc.sync.dma_start(dqt[:, 0, :], QTz[0][2][:])
                nc.sync.dma_start(dqt[:, 1, :], QTz[0][3][:])
                dkt = dbg_kt.rearrange("p (i m) -> p i m", i=4)
                nc.sync.dma_start(dkt[:, 0:2, :], KT_sb[2][:])
                nc.sync.dma_start(dkt[:, 2:4, :], KT_sb[3][:])

    nc.compile()
    return nc


def make_in_maps(x, Wq, bq, Wkv, bkv, Wo, bo):
    import ml_dtypes
    BF = ml_dtypes.bfloat16

    x = np.asarray(x, np.float32)
    Wq = np.asarray(Wq, np.float32)
    bq = np.asarray(bq, np.float32)
    Wkv = np.asarray(Wkv, np.float32)
    bkv = np.asarray(bkv, np.float32)
    Wo = np.asarray(Wo, np.float32)

    Wk, Wv = Wkv[:D], Wkv[D:]
    bk, bv = bkv[:D], bkv[D:]

    # mask[kk, u] = 1 iff u >= kk + 384 ; slice [384-r : 896-r] gives the
    # keep-mask (q >= k + r) for a diagonal chunk with offset r.
    u = np.arange(896)[None, :]
    kk = np.arange(128)[:, None]
    mask = (u >= kk + 384).astype(BF)

    in_maps = []
    for c in range(NCORES):
        b = c // (NCORES // B)
        hs = HPC * (c % (NCORES // B))
        rows = slice(hs * HD, hs * HD + HPC * HD)
        # SCALE folded into Wq/bq on host; sclz is a pure 0/1 row mask.
        bq_c = bq[rows] * SCALE
        bqz = np.zeros((128, HPC), np.float32)
        sclz = np.zeros((128, HPC), np.float32)
        for h in range(HPC):
            po = (h % 2) * 64
            m = h // 2
            bqz[po:po + 64, h] = bq_c[m * 128 + po:m * 128 + po + 64]
            sclz[po:po + 64, h] = 1.0
        # WoP: head pairs stacked per 128 partitions, [128, 2, 1024]
        wo_c = np.ascontiguousarray(Wo[:, rows].T)          # [256, 1024]
        woP = wo_c.reshape(2, 128, D).transpose(1, 0, 2)    # [128, 2, 1024]
        # partition-major packed loads: [p][...] contiguous per partition
        def pmaj(w):      # [1024, 256] -> [128, 8, 256]
            return w.reshape(DCH, 128, HPC * HD).transpose(1, 0, 2)
        wqk_h = np.concatenate(
            [pmaj(Wq[rows].T * SCALE)[:, None], pmaj(Wk[rows].T)[:, None]],
            axis=1).reshape(128, 2 * DCH * 256)
        wvo_h = np.concatenate(
            [pmaj(Wv[rows].T).reshape(128, DCH * 256),
             woP.reshape(128, 2 * D)], axis=1)
        xr = x[b].T.reshape(DCH, 128, N).transpose(1, 0, 2)  # [128, 8, 2048]
        in_maps.append({
            "xb0": np.ascontiguousarray(
                xr[:, :, :QTILE].reshape(128, DCH * QTILE)).astype(BF),
            "xrest": np.ascontiguousarray(
                xr[:, :, QTILE:].reshape(128, DCH * 3 * QTILE)).astype(BF),
            "wqk": np.ascontiguousarray(wqk_h).astype(BF),
            "wvo": np.ascontiguousarray(wvo_h).astype(BF),
            "bqz": bqz,
            "sclz": sclz,
            "bk": np.ascontiguousarray(bk[rows]),
            "mask": mask,
        })
    return in_maps


_NC_CACHE = None


def _get_nc():
    global _NC_CACHE
    if _NC_CACHE is None:
        _NC_CACHE = build_kernel()
    return _NC_CACHE


def kernel(x, Wq, bq, Wkv, bkv, Wo, bo, _trace=False, _trace_kwargs=None):
    nc = _get_nc()
    in_maps = make_in_maps(x, Wq, bq, Wkv, bkv, Wo, bo)
    kwargs = {}
    if _trace:
        kwargs = dict(trace=True, trace_cores=list(range(NCORES)),
                      **(_trace_kwargs or {}))
    res = run_bass_kernel_spmd(nc, in_maps, core_ids=list(range(NCORES)),
                               **kwargs)
    out = np.zeros((B, N, D), np.float32)
    for c, r in enumerate(res.results):
        out[c // (NCORES // B)] += r["y"]
    bv = np.asarray(bkv, np.float32)[D:]
    Wo_f = np.asarray(Wo, np.float32)
    out += (np.asarray(bo, np.float32) + bv @ Wo_f.T)[None, None, :]
    if _trace:
        kernel.last_results = res
    return out


# revision 42
# speedup vs baseline: 1.1565x; 1.1565x over previous
"""Trainium2 Bass kernel for causal multi-head attention (b=2, n=2048, d=1024, h=16).

Sharding: 8 cores; core c handles batch (c // 4) and the 4 heads
[4*(c%4), 4*(c%4)+4).  Each core computes its heads' attention plus its
partial output projection y_part = O_heads @ Wo[:, cols].T ; the host sums
the 4 per-batch partials and adds bo + bv @ Wo.T (the V-bias contribution
commutes with the softmax average, so it never goes on device).

All matmuls run in bf16 (measured: bf16@512 issues at ~259 ns vs f32r's
~285-500 ns, and narrow f32r matmuls fall off a cliff).  fp32 PSUM
accumulation keeps the error at ~3.5e-3 (CPU-simulated) vs the 2e-2 gate.

On-device pipeline per core, interleaved per 512-token block:
  xT (d-major, bf16) -> QT,KT [hd, n] and V [n, hd] projections
  ST tile [k,q] = KT-chunk.T x QTz_h      (bf16, K=128: both heads' KT rows
                                           with the other head's QT rows
                                           zeroed; 1/8 scale folded into Wq)
  PT = exp(ST) in bf16 (no max subtraction; fp32-exp safe)
  causal masking by multiplying diagonal-band tiles with a bf16 0/1 mask
  OT [h%2 ? 64:128 rows, q] += V_aug-chunk.T x PT ; V_aug carries a ones
      column so the softmax denominator l lands at partition 64 (even
      heads) / partition 0 (odd heads), next to OT's 64 rows.
  rl = 1/l (one-row DVE reciprocal), broadcast across partitions with a
      single SBUF->SBUF DMA (no DRAM bounce)
  OTall[(h%2)*64:+64, h//2, q] = OT * rl   (pair-packed, bf16)
  y[tok, :] += OTall-pair-chunk.T x WoP    (bf16, K=128 dense: head pairs
      stacked, so no zero-row padding in the output projection)
"""

import numpy as np

import concourse.bass as bass
import concourse.mybir as mybir
import concourse.tile as tile
from concourse import bacc
from concourse.bass_utils import run_bass_kernel_spmd

D = 1024          # d_model
N = 2048          # sequence length
B = 2             # batch
H_TOT = 16        # total heads
HD = 64           # head dim
HPC = 4           # heads per core
NCORES = 8
SCALE = HD ** -0.5

F32 = mybir.dt.float32
BF16 = mybir.dt.bfloat16

QTILE = 512       # q-tile width (free dim of score matmuls)
KCH = 128         # k-chunk (partition dim of score tiles)
NQT = N // QTILE  # 4
NKC = N // KCH    # 16
DCH = D // 128    # 8 d_model chunks
NJUNK = 22        # PE warm-up matmuls
DEBUG_DUMP = False  # add debug DRAM outputs (OTall, l, 1/l)


def build_kernel():
    nc = bacc.Bacc("TRN2", target_bir_lowering=False, debug=False,
                   num_devices=NCORES)

    # inputs pre-tiled on host into partition-major layouts so each load is
    # one DMA with fat (multi-KB) per-partition contiguous descriptors
    xb0 = nc.dram_tensor("xb0", [128, DCH * QTILE], BF16,
                         kind="ExternalInput").ap()
    xrest = nc.dram_tensor("xrest", [128, DCH * 3 * QTILE], BF16,
                           kind="ExternalInput").ap()
    wqk = nc.dram_tensor("wqk", [128, 2 * DCH * 256], BF16,
                         kind="ExternalInput").ap()
    wvo = nc.dram_tensor("wvo", [128, DCH * 256 + 2 * D], BF16,
                         kind="ExternalInput").ap()
    bqz = nc.dram_tensor("bqz", [128, HPC], F32, kind="ExternalInput").ap()
    sclz = nc.dram_tensor("sclz", [128, HPC], F32, kind="ExternalInput").ap()
    bkd = nc.dram_tensor("bk", [HPC * HD], F32, kind="ExternalInput").ap()
    maskd = nc.dram_tensor("mask", [128, 896], BF16, kind="ExternalInput").ap()
    y = nc.dram_tensor("y", [N, D], F32, kind="ExternalOutput").ap()
    if DEBUG_DUMP:
        dbg_ot = nc.dram_tensor("dbg_ot", [128, 2 * N], BF16,
                                kind="ExternalOutput").ap()
        dbg_l = nc.dram_tensor("dbg_l", [16, 512], F32,
                               kind="ExternalOutput").ap()
        dbg_rl = nc.dram_tensor("dbg_rl", [16, 512], F32,
                                kind="ExternalOutput").ap()
        dbg_xk = nc.dram_tensor("dbg_xk", [128, 8 * 512], BF16,
                                kind="ExternalOutput").ap()
        dbg_qt = nc.dram_tensor("dbg_qt", [128, 2 * 512], BF16,
                                kind="ExternalOutput").ap()
        dbg_kt = nc.dram_tensor("dbg_kt", [128, 4 * 512], BF16,
                                kind="ExternalOutput").ap()

    Exp = mybir.ActivationFunctionType.Exp
    Identity = mybir.ActivationFunctionType.Identity

    with tile.TileContext(nc) as tc:
        from contextlib import ExitStack
        with ExitStack() as ctx:
            singles = ctx.enter_context(tc.tile_pool(name="singles", bufs=1))
            pt_pool = ctx.enter_context(tc.tile_pool(name="pt", bufs=4))
            r_pool = ctx.enter_context(tc.tile_pool(name="rp", bufs=2))
            yout = ctx.enter_context(tc.tile_pool(name="yout", bufs=3))
            dram = ctx.enter_context(
                tc.tile_pool(name="dram", bufs=2, space="DRAM"))
            ps_mm = ctx.enter_context(
                tc.tile_pool(name="psmm", bufs=2, space="PSUM"))
            ps_st = ctx.enter_context(
                tc.tile_pool(name="psst", bufs=2, space="PSUM"))
            ps_ot = ctx.enter_context(
                tc.tile_pool(name="psot", bufs=2, space="PSUM"))

            # --- resident inputs (DMA order = priority: first MMs need
            # wq/wk + x block 0) -------------------------------------------
            wqk_r = wqk.rearrange("p (t o m) -> p t o m", t=2, o=DCH)
            wqk_sb = singles.tile([128, 2, DCH, 256], BF16)
            nc.sync.dma_start(wqk_sb[:, 0], wqk_r[:, 0])
            wq_sb = wqk_sb[:, 0]
            wk_sb = wqk_sb[:, 1]
            bqz_sb = singles.tile([128, HPC], F32)
            nc.sync.dma_start(bqz_sb[:], bqz)
            sclz_sb = singles.tile([128, HPC], F32)
            nc.sync.dma_start(sclz_sb[:], sclz)
            bk_sb = singles.tile([128, 2], F32)
            nc.sync.dma_start(bk_sb[:], bkd.rearrange("(o p) -> p o", p=128))

            # x lives in one [128, k-chunk, token] tile; block 0 lands first
            # so compute can start early.
            XKA = singles.tile([128, DCH, N], BF16, name="xka")
            xk = [[XKA[:, k, b * QTILE:(b + 1) * QTILE] for b in range(NQT)]
                  for k in range(DCH)]
            nc.sync.dma_start(
                XKA[:, :, 0:QTILE],
                xb0.rearrange("p (o m) -> p o m", o=DCH))
            nc.sync.dma_start(wqk_sb[:, 1], wqk_r[:, 1])

            wvo_sb = singles.tile([128, DCH * 256 + 2 * D], BF16)
            nc.sync.dma_start(wvo_sb[:, :DCH * 256], wvo[:, :DCH * 256])
            wv_sb = wvo_sb[:, :DCH * 256].rearrange("p (o m) -> p o m", o=DCH)
            wop_sb = wvo_sb[:, DCH * 256:].rearrange("p (t m) -> p t m", t=2)

            nc.sync.dma_start(
                XKA[:, :, QTILE:],
                xrest.rearrange("p (o m) -> p o m", o=DCH))
            nc.sync.dma_start(wvo_sb[:, DCH * 256:], wvo[:, DCH * 256:])
            mask_sb = singles.tile([128, 896], BF16)
            nc.sync.dma_start(mask_sb[:], maskd)

            # PE warm-up: the first ~8us are DMA-bound with the PE idle,
            # which leaves the PE clock throttled to 1.2 GHz when real work
            # starts.  Issue dependency-free junk matmuls so the activity
            # monitor unthrottles before the first projection matmul.
            junk = singles.tile([128, 512], BF16)
            nc.vector.memset(junk[:], 0.0)
            ones1 = singles.tile([1, 128], BF16, name="ones1")
            nc.vector.memset(ones1[:], 1.0)
            for i in range(NJUNK):
                wps = ps_ot.tile([128, 512], F32, tag="ot", name="wps")
                nc.tensor.matmul(wps[:], lhsT=junk[:, :128], rhs=junk[:],
                                 start=True, stop=True)

            # V_aug layout per head (128 cols each inside V_sb):
            #   even h: cols 0-63 = V, col 64 = ones  -> l at partition 64
            #   odd h:  col 0 = ones, cols 64-127 = V -> l at partition 0
            # (cols 65-127 even / 1-63 odd zeroed so stray PSUM rows stay 0)
            QTz = [[singles.tile([128, QTILE], BF16, name=f"qtz{h}_{i}")
                    for i in range(NQT)] for h in range(HPC)]
            KT_sb = [singles.tile([128, 2, QTILE], BF16, name=f"kt{i}")
                     for i in range(NQT)]
            V_sb = [singles.tile([128, 4, HPC * 128], BF16, name=f"v{i}")
                    for i in range(NQT)]
            OTall = singles.tile([128, 2, N], BF16, name="otall")
            for blk in range(NQT):
                nc.gpsimd.memset(V_sb[blk][:], 0.0)
                for h in range(HPC):
                    oc = h * 128 + (64 if h % 2 == 0 else 0)
                    nc.gpsimd.memset(V_sb[blk][:, :, oc:oc + 1], 1.0)

            def project(blk):
                for m in range(2):
                    ps = ps_mm.tile([128, 512], F32, tag="mm")
                    for k in range(DCH):
                        nc.tensor.matmul(
                            ps[:],
                            lhsT=wq_sb[:, k, m * 128:(m + 1) * 128],
                            rhs=xk[k][blk][:],
                            start=(k == 0), stop=(k == DCH - 1))
                    for hh in range(2):
                        h = 2 * m + hh
                        # per-partition scale zeroes the other head's rows
                        nc.scalar.activation(
                            QTz[h][blk][:], ps[:], Identity,
                            bias=bqz_sb[:, h:h + 1],
                            scale=sclz_sb[:, h:h + 1])
                for m in range(2):
                    ps = ps_mm.tile([128, 512], F32, tag="mm")
                    for k in range(DCH):
                        nc.tensor.matmul(
                            ps[:],
                            lhsT=wk_sb[:, k, m * 128:(m + 1) * 128],
                            rhs=xk[k][blk][:],
                            start=(k == 0), stop=(k == DCH - 1))
                    nc.scalar.activation(
                        KT_sb[blk][:, m, :], ps[:], Identity,
                        bias=bk_sb[:, m:m + 1], scale=1.0)
                for tt in range(4):
                    ps = ps_mm.tile([128, 512], F32, tag="mm")
                    for k in range(DCH):
                        nc.tensor.matmul(
                            ps[:, :HPC * HD],
                            lhsT=xk[k][blk][:, tt * 128:(tt + 1) * 128],
                            rhs=wv_sb[:, k, :],
                            start=(k == 0), stop=(k == DCH - 1))
                    for h in range(HPC):
                        vc = h * 128 + (0 if h % 2 == 0 else 64)
                        # split casts across ACT and DVE so neither queue
                        # rate-limits the projection PSUM rotation
                        if (tt + h) % 2 == 0:
                            nc.scalar.activation(
                                V_sb[blk][:, tt, vc:vc + HD],
                                ps[:, h * HD:(h + 1) * HD], Identity)
                        else:
                            nc.vector.tensor_copy(
                                V_sb[blk][:, tt, vc:vc + HD],
                                ps[:, h * HD:(h + 1) * HD])

            def attend(qi, carry):
                # carry: pending normalize-finish thunks (recip+mul) whose
                # broadcast DMAs are still in flight.  Each is flushed two
                # heads later so the DVE never stalls on the DMA bounce.
                q0 = qi * QTILE
                for h in range(HPC):
                    mi = h // 2
                    po = (h % 2) * 64          # OT partition offset
                    vbase = h * 128
                    nrows = 65 if h % 2 == 0 else 128
                    lrow = 64 if h % 2 == 0 else 0
                    nprs = 2 * (qi + 1)        # pairs of 128-k-chunks
                    pso = ps_ot.tile([128, 512], F32, tag="ot", name="pso")

                    pair_ps = [None] * nprs
                    pair_pt = [None] * nprs

                    def consume(pi):
                        # last pair of each q-tile: only columns q >= r are
                        # unmasked (r = 256, 384); compute just those.
                        shrunk = (pi == nprs - 1)
                        pss, pt = pair_ps[pi], pair_pt[pi]
                        if shrunk:
                            for j in range(2):
                                r = (2 * pi + j) * KCH - q0
                                nc.scalar.activation(
                                    pt[:, j, r:], pss[:, j, r:], Exp)
                        else:
                            nc.scalar.activation(pt[:], pss[:], Exp)
                        for j in range(2):
                            ki = 2 * pi + j
                            r = ki * KCH - q0
                            if r >= 0:
                                s = max(r, 0)
                                nc.vector.tensor_mul(
                                    pt[:, j, s:], pt[:, j, s:],
                                    mask_sb[:, 384 - r + s:384 - r + 512])
                        for j in range(2):
                            ki = 2 * pi + j
                            s = max(ki * KCH - q0, 0)
                            nc.tensor.matmul(
                                pso[:nrows, s:],
                                lhsT=V_sb[ki // 4][:, ki % 4,
                                                   vbase:vbase + nrows],
                                rhs=pt[:, j, s:],
                                start=(ki == 0), stop=(ki == 4 * (qi + 1) - 1))

                    for pi in range(nprs):
                        pss = ps_st.tile([128, 2, 512], F32, tag="st",
                                         name="pss")
                        pair_ps[pi] = pss
                        pair_pt[pi] = pt_pool.tile([128, 2, 512], BF16,
                                                   tag="pt", name="pt")
                        for j in range(2):
                            ki = 2 * pi + j
                            s = (ki * KCH - q0) if pi == nprs - 1 else 0
                            nc.tensor.matmul(
                                pss[:, j, s:],
                                lhsT=KT_sb[ki // 4][:, mi,
                                                    (ki % 4) * 128:
                                                    (ki % 4) * 128 + 128],
                                rhs=QTz[h][qi][:, s:],
                                start=True, stop=True)
                        if pi > 0:
                            consume(pi - 1)
                    consume(nprs - 1)

                    # normalize, pipelined: drain now (stage OT+l off PSUM,
                    # 1/l bounce through DRAM — SBUF DMA sources reject zero
                    # partition stride), finish (recip+mul) two heads later
                    # so the DVE queue never waits on the bounce round trip.
                    # reciprocal ops must START at partition 0 (offset-64
                    # reciprocal returns garbage on HW), so the recip-vs-
                    # broadcast order flips with head parity.
                    if len(carry) == 2:
                        carry.pop(0)()
                    otu = r_pool.tile([128, 512], F32, tag="otu", name="otu")
                    nc.vector.tensor_copy(otu[:nrows, :], pso[:nrows, :])
                    sc = dram.tile([1, 512], F32, tag="sc", name="sc")
                    rb = r_pool.tile([128, 512], F32, tag="rb", name="rb")
                    if h % 2 == 0:
                        # l at partition 64: broadcast raw l, then recip the
                        # 64 partitions we need (offset 0).
                        nc.sync.dma_start(sc[:], otu[lrow:lrow + 1, :])
                        row = sc[0, :]
                        bcast = bass.AP(tensor=row.tensor, offset=row.offset,
                                        ap=[[0, HD]] + list(row.ap))
                        nc.sync.dma_start(rb[:HD, :], bcast)

                        def finish(otu=otu, rb=rb, po=po, mi=mi):
                            nc.vector.reciprocal_approx_fast(
                                out=rb[:HD, :], in_=rb[:HD, :])
                            nc.vector.tensor_mul(
                                OTall[po:po + HD, mi, q0:q0 + QTILE],
                                otu[po:po + HD, :], rb[po:po + HD, :])
                    elif qi == NQT - 1 and h == HPC - 1:
                        # very last head: the DMA bounce's ~5us latency would
                        # sit on the critical path before the final out-proj.
                        # Broadcast 1/l with a K=1 ones-matmul on the PE and
                        # drain it through ACT instead (~1.5us).
                        rl = r_pool.tile([128, 512], F32, tag="rl", name="rl")
                        nc.vector.reciprocal_approx_fast(
                            out=rl[0:1, :], in_=otu[0:1, :])
                        rlb = r_pool.tile([128, 512], BF16, tag="rlb",
                                          name="rlb")
                        nc.vector.tensor_copy(rlb[0:1, :], rl[0:1, :])
                        rb_ps = ps_mm.tile([128, 512], F32, tag="mm",
                                           name="rbps")
                        nc.tensor.matmul(rb_ps[:], lhsT=ones1[0:1, :],
                                         rhs=rlb[0:1, :], start=True,
                                         stop=True)
                        nc.scalar.activation(rb[:], rb_ps[:], Identity)

                        def finish(otu=otu, rb=rb, po=po, mi=mi):
                            nc.vector.tensor_mul(
                                OTall[po:po + HD, mi, q0:q0 + QTILE],
                                otu[po:po + HD, :], rb[po:po + HD, :])
                    else:
                        # l at partition 0: recip the single row first, then
                        # broadcast 1/l.
                        rl = r_pool.tile([128, 512], F32, tag="rl", name="rl")
                        nc.vector.reciprocal_approx_fast(
                            out=rl[0:1, :], in_=otu[0:1, :])
                        nc.sync.dma_start(sc[:], rl[0:1, :])
                        row = sc[0, :]
                        bcast = bass.AP(tensor=row.tensor, offset=row.offset,
                                        ap=[[0, HD]] + list(row.ap))
                        nc.sync.dma_start(rb[HD:, :], bcast)

                        def finish(otu=otu, rb=rb, po=po, mi=mi):
                            nc.vector.tensor_mul(
                                OTall[po:po + HD, mi, q0:q0 + QTILE],
                                otu[po:po + HD, :], rb[po:po + HD, :])
                    carry.append(finish)
                    if DEBUG_DUMP:
                        di = h * 4 + qi
                        nc.sync.dma_start(dbg_l[di:di + 1, :],
                                          otu[lrow:lrow + 1, :])
                        nc.sync.dma_start(dbg_rl[di:di + 1, :],
                                          rb[po:po + 1, :])

            def outproj(qi):
                # output projection for this q-block's 4 token chunks
                q0 = qi * QTILE
                for tt in range(4):
                    t0 = q0 + tt * 128
                    for half in range(2):
                        # psy shares ps_mm with the projections ON PURPOSE:
                        # the pool rotation keeps project(qi+2) queued behind
                        # this outproj, so the scheduler cannot hoist all
                        # projection work early and leave the PE bare during
                        # the attend tail's normalize bounce.
                        if qi == NQT - 1 and (tt * 2 + half) % 2 == 1:
                            ps = ps_st.tile([128, 512], F32, tag="st",
                                            name="psy")
                        else:
                            ps = ps_mm.tile([128, 512], F32, tag="mm",
                                            name="psy")
                        for m in range(2):
                            nc.tensor.matmul(
                                ps[:],
                                lhsT=OTall[:, m, t0:t0 + 128],
                                rhs=wop_sb[:, m, half * 512:half * 512 + 512],
                                start=(m == 0), stop=(m == 1))
                        yt = yout.tile([128, 512], F32, tag="y", name="yt")
                        # split drain copies across DVE and ACT so neither
                        # engine rate-limits the out-proj PSUM rotation
                        if half == 0:
                            nc.vector.tensor_copy(yt[:], ps[:])
                        else:
                            nc.scalar.activation(yt[:], ps[:], Identity)
                        nc.sync.dma_start(
                            y[t0:t0 + 128, half * 512:half * 512 + 512],
                            yt[:])

            # software pipeline: emit outproj(qi) after project(qi+1) so the
            # PE covers the last head's normalize latency (DMA bounce) with
            # the next block's projection matmuls instead of stalling.
            pend = []
            for blk in range(NQT):
                project(blk)
                if blk > 0:
                    for f in pend:
                        f()
                    pend = []
                    outproj(blk - 1)
                attend(blk, pend)
            for f in pend:
                f()
            outproj(NQT - 1)
            if DEBUG_DUMP:
                nc.sync.dma_start(
                    dbg_ot.rearrange("p (t m) -> p t m", t=2), OTall[:])
                dxk = dbg_xk.rearrange("p (i m) -> p i m", i=8)
                for i, (k, b) in enumerate(
                        [(0, 2), (0, 3), (3, 2), (3, 3),
                         (5, 2), (5, 3), (7, 2), (7, 3)]):
                    nc.sync.dma_start(dxk[:, i, :], xk[k][b][:])
                dqt = dbg_qt.rearrange("p (i m) -> p i m", i=2)
                nc.sync.dma_start(dqt[:, 0, :], QTz[0][2][:])
                nc.sync.dma_start(dqt[:, 1, :], QTz[0][3][:])
                dkt = dbg_kt.rearrange("p (i m) -> p i m", i=4)
                nc.sync.dma_start(dkt[:, 0:2, :], KT_sb[2][:])
                nc.sync.dma_start(dkt[:, 2:4, :], KT_sb[3][:])

    nc.compile()
    return nc


def make_in_maps(x, Wq, bq, Wkv, bkv, Wo, bo):
    import ml_dtypes
    BF = ml_dtypes.bfloat16

    x = np.asarray(x, np.float32)
    Wq = np.asarray(Wq, np.float32)
    bq = np.asarray(bq, np.float32)
    Wkv = np.asarray(Wkv, np.float32)
    bkv = np.asarray(bkv, np.float32)
    Wo = np.asarray(Wo, np.float32)

    Wk, Wv = Wkv[:D], Wkv[D:]
    bk, bv = bkv[:D], bkv[D:]

    # mask[kk, u] = 1 iff u >= kk + 384 ; slice [384-r : 896-r] gives the
    # keep-mask (q >= k + r) for a diagonal chunk with offset r.
    u = np.arange(896)[None, :]
    kk = np.arange(128)[:, None]
    mask = (u >= kk + 384).astype(BF)

    in_maps = []
    for c in range(NCORES):
        b = c // (NCORES // B)
        hs = HPC * (c % (NCORES // B))
        rows = slice(hs * HD, hs * HD + HPC * HD)
        # SCALE folded into Wq/bq on host; sclz is a pure 0/1 row mask.
        bq_c = bq[rows] * SCALE
        bqz = np.zeros((128, HPC), np.float32)
        sclz = np.zeros((128, HPC), np.float32)
        for h in range(HPC):
            po = (h % 2) * 64
            m = h // 2
            bqz[po:po + 64, h] = bq_c[m * 128 + po:m * 128 + po + 64]
            sclz[po:po + 64, h] = 1.0
        # WoP: head pairs stacked per 128 partitions, [128, 2, 1024]
        wo_c = np.ascontiguousarray(Wo[:, rows].T)          # [256, 1024]
        woP = wo_c.reshape(2, 128, D).transpose(1, 0, 2)    # [128, 2, 1024]
        # partition-major packed loads: [p][...] contiguous per partition
        def pmaj(w):      # [1024, 256] -> [128, 8, 256]
            return w.reshape(DCH, 128, HPC * HD).transpose(1, 0, 2)
        wqk_h = np.concatenate(
            [pmaj(Wq[rows].T * SCALE)[:, None], pmaj(Wk[rows].T)[:, None]],
            axis=1).reshape(128, 2 * DCH * 256)
        wvo_h = np.concatenate(
            [pmaj(Wv[rows].T).reshape(128, DCH * 256),
             woP.reshape(128, 2 * D)], axis=1)
        xr = x[b].T.reshape(DCH, 128, N).transpose(1, 0, 2)  # [128, 8, 2048]
        in_maps.append({
            "xb0": np.ascontiguousarray(
                xr[:, :, :QTILE].reshape(128, DCH * QTILE)).astype(BF),
            "xrest": np.ascontiguousarray(
                xr[:, :, QTILE:].reshape(128, DCH * 3 * QTILE)).astype(BF),
            "wqk": np.ascontiguousarray(wqk_h).astype(BF),
            "wvo": np.ascontiguousarray(wvo_h).astype(BF),
            "bqz": bqz,
            "sclz": sclz,
            "bk": np.ascontiguousarray(bk[rows]),
            "mask": mask,
        })
    return in_maps


_NC_CACHE = None


def _get_nc():
    global _NC_CACHE
    if _NC_CACHE is None:
        _NC_CACHE = build_kernel()
    return _NC_CACHE


def kernel(x, Wq, bq, Wkv, bkv, Wo, bo, _trace=False, _trace_kwargs=None):
    nc = _get_nc()
    in_maps = make_in_maps(x, Wq, bq, Wkv, bkv, Wo, bo)
    kwargs = {}
    if _trace:
        kwargs = dict(trace=True, trace_cores=list(range(NCORES)),
                      **(_trace_kwargs or {}))
    res = run_bass_kernel_spmd(nc, in_maps, core_ids=list(range(NCORES)),
                               **kwargs)
    out = np.zeros((B, N, D), np.float32)
    for c, r in enumerate(res.results):
        out[c // (NCORES // B)] += r["y"]
    bv = np.asarray(bkv, np.float32)[D:]
    Wo_f = np.asarray(Wo, np.float32)
    out += (np.asarray(bo, np.float32) + bv @ Wo_f.T)[None, None, :]
    if _trace:
        kernel.last_results = res
    return out
